# revision 5
# baseline (speedup 1.0000x reference)
"""Axial attention (attention along W only) Trainium2 Bass kernel.

Reference computation (per batch b, row h):
    q = x @ wq; k = x @ wk; v = x @ wv          (x = inputs[b,h] in [W, C])
    s = q @ k^T ; p = softmax(s, axis=-1)
    out[b,h] = (p @ v) @ wv^T

Key algebraic fusion (collapses DIM entirely):
    s   = x @ (wq @ wk^T) @ x^T = x @ Wqk @ x^T
    out = p @ x @ (wv @ wv^T)   = p @ x @ Wvv
Wqk/Wvv are [C, C] host-precomputed weight products (one-time 256^3 matmuls).

Softmax without per-row max subtraction (constant bias -30 instead): scores are
~N(0, 10^2); fp32 exp is safe to |s| ~ 85, so overflow probability is nil and
the constant shift cancels exactly in the normalization.

Sharding: data-parallel over batch B=8 -> one batch per NeuronCore. Weights
replicated. Each core processes 128 independent (h) tiles of [W=128, C=256].

Per h-tile dataflow on-chip (PE convention: matmul(out, lhsT, rhs) = lhsT.T @ rhs):
    xT  = DMA-transpose load of x (bf16, 2 chunks of [128c, 128w])
    tT[c2,i] = sum_c Wqk[c,c2] xT[c,i]            4 MMs  (t = x @ Wqk, transposed)
    s[i,j]   = sum_c2 tT[c2,i] xT[c2,j]           2 MMs
    sT[j,i]  = sum_c2 xT[c2,j] tT[c2,i]           2 MMs  (same values, transposed)
    exp_s = exp(s-30) with accum_out -> Z[i]      ACT (row sums for softmax)
    exp_sT = exp(sT-30)                           ACT (the actual weights, bf16)
    uT[c,i]  = sum_j x[j,c] exp_sT[j,i]           2 MMs  (u = p~ @ x, transposed)
    o[i,c']  = sum_c uT[c,i] Wvv[c,c']            2 MMs
    out = o * (1/Z[i])                            ACT copy+scale, fp32
"""

import os

import numpy as np
import ml_dtypes

B, H, W, C = 8, 128, 128, 256
N_CORES = 8

_PROGRAM = None  # cached (nc,) build


def _build_program(n_htiles=H, repeat=1):
    import concourse.bass as bass
    import concourse.mybir as mybir
    import concourse.tile as tile
    from concourse import bacc
    from concourse.bass import ts

    bf16 = mybir.dt.bfloat16
    f32 = mybir.dt.float32

    nc = bacc.Bacc(
        "TRN2",
        target_bir_lowering=False,
        debug=False,
        enable_asserts=False,
        num_devices=N_CORES,
    )

    x_d = nc.dram_tensor("x", [n_htiles, W, C], bf16, kind="ExternalInput").ap()
    wqk_d = nc.dram_tensor("wqk", [C, C], bf16, kind="ExternalInput").ap()
    wvv_d = nc.dram_tensor("wvv", [C, C], bf16, kind="ExternalInput").ap()
    out_d = nc.dram_tensor("out", [n_htiles, W, C], f32, kind="ExternalOutput").ap()

    with tile.TileContext(nc) as tc:
        with (
            tc.tile_pool(name="singles", bufs=1) as singles,
            tc.tile_pool(name="work", bufs=4) as work,
            tc.tile_pool(name="psum", bufs=2, space="PSUM") as psum,
        ):
            # Replicated weights, loaded once. Layout [p, chunk, col]:
            # wqk_s[p, o, c2] = Wqk[o*128 + p, c2]
            wqk_s = singles.tile([128, 2, C], bf16)
            nc.sync.dma_start(wqk_s, wqk_d.rearrange("(o p) c -> p o c", p=128))
            wvv_s = singles.tile([128, 2, C], bf16)
            nc.sync.dma_start(wvv_s, wvv_d.rearrange("(o p) c -> p o c", p=128))
            bias_s = singles.tile([128, 1], f32)
            nc.vector.memset(bias_s, -30.0)

            for h in range(n_htiles * repeat):
                h = h % n_htiles
                # ---- loads ----
                x_s = work.tile([W, C], bf16, tag="x")
                nc.sync.dma_start(x_s, x_d[h])
                xT_s = work.tile([128, 2, W], bf16, tag="xT")
                for c in range(2):
                    nc.sync.dma_start_transpose(xT_s[:, c, :], x_d[h, :, ts(c, 128)])

                # ---- psum tiles (packed to stay within 8 banks at bufs=2) ----
                ps_ts = psum.tile([128, 384], f32, tag="ps_ts")  # tT @ 0:256, s @ 256:384
                ps_su = psum.tile([128, 384], f32, tag="ps_su")  # sT @ 0:128, uT @ 128:384
                ps_o = psum.tile([128, 256], f32, tag="ps_o")

                # ---- tT = (x @ Wqk)^T : [c2, i] ----
                for j in range(2):  # c2 chunk
                    for o in range(2):  # contraction c chunk
                        nc.tensor.matmul(
                            ps_ts[:, ts(j, 128)],
                            lhsT=wqk_s[:, o, ts(j, 128)],
                            rhs=xT_s[:, o, :],
                            start=(o == 0),
                            stop=(o == 1),
                        )
                tT_s = work.tile([128, 256], bf16, tag="tT")
                nc.vector.tensor_copy(tT_s, ps_ts[:, 0:256])

                # ---- s [i,j] and sT [j,i] ----
                for j in range(2):
                    nc.tensor.matmul(
                        ps_ts[:, 256:384],
                        lhsT=tT_s[:, ts(j, 128)],
                        rhs=xT_s[:, j, :],
                        start=(j == 0),
                        stop=(j == 1),
                    )
                for j in range(2):
                    nc.tensor.matmul(
                        ps_su[:, 0:128],
                        lhsT=xT_s[:, j, :],
                        rhs=tT_s[:, ts(j, 128)],
                        start=(j == 0),
                        stop=(j == 1),
                    )

                # ---- softmax pieces ----
                z_s = work.tile([128, 1], f32, tag="z")
                exps_scr = work.tile([128, 128], bf16, tag="exps")
                nc.scalar.activation(
                    out=exps_scr,
                    in_=ps_ts[:, 256:384],
                    func=mybir.ActivationFunctionType.Exp,
                    bias=bias_s,
                    scale=1.0,
                    accum_out=z_s,
                )
                rz_s = work.tile([128, 1], f32, tag="rz")
                nc.vector.reciprocal(rz_s, z_s)
                expsT_s = work.tile([128, 128], bf16, tag="expsT")
                nc.scalar.activation(
                    out=expsT_s,
                    in_=ps_su[:, 0:128],
                    func=mybir.ActivationFunctionType.Exp,
                    bias=bias_s,
                    scale=1.0,
                )

                # ---- uT [c, i] = x^T @ exp_sT ----
                for o in range(2):
                    nc.tensor.matmul(
                        ps_su[:, 128 + 128 * o : 256 + 128 * o],
                        lhsT=x_s[:, ts(o, 128)],
                        rhs=expsT_s,
                        start=True,
                        stop=True,
                    )
                uT_s = work.tile([128, 256], bf16, tag="uT")
                nc.vector.tensor_copy(uT_s, ps_su[:, 128:384])

                # ---- o [i, c'] = u @ Wvv ----
                for o in range(2):
                    nc.tensor.matmul(
                        ps_o,
                        lhsT=uT_s[:, ts(o, 128)],
                        rhs=wvv_s[:, o, :],
                        start=(o == 0),
                        stop=(o == 1),
                    )

                # ---- normalize + store ----
                out_s = work.tile([128, 256], f32, tag="out")
                nc.scalar.activation(
                    out=out_s,
                    in_=ps_o,
                    func=mybir.ActivationFunctionType.Identity,
                    bias=0.0,
                    scale=rz_s,
                )
                nc.gpsimd.dma_start(out_d[h], out_s)

    nc.finalize()
    return nc


def _get_program():
    global _PROGRAM
    if _PROGRAM is None:
        _PROGRAM = _build_program()
    return _PROGRAM


LAST_RESULT = None  # BassKernelResults of the most recent kernel() call


def kernel(inputs, wq, wk, wv):
    """Full-input entry point: inputs [8,128,128,256] f32, wq/wk/wv [256,256] f32.
    Returns [8,128,128,256] f32."""
    global LAST_RESULT
    from concourse.bass_utils import run_bass_kernel_spmd

    bf16 = ml_dtypes.bfloat16
    wqk = (wq.astype(np.float32) @ wk.astype(np.float32).T).astype(bf16)
    wvv = (wv.astype(np.float32) @ wv.astype(np.float32).T).astype(bf16)
    x_bf = np.ascontiguousarray(inputs.astype(bf16))

    in_maps = [
        {"x": x_bf[b], "wqk": wqk, "wvv": wvv} for b in range(N_CORES)
    ]

    nc = _get_program()
    trace = bool(int(os.environ.get("AXIAL_TRACE", "0")))
    try:
        res = run_bass_kernel_spmd(
            nc,
            in_maps,
            core_ids=list(range(N_CORES)),
            trace=trace,
        )
    except ModuleNotFoundError:
        if not trace:
            raise
        res = run_bass_kernel_spmd(
            nc, in_maps, core_ids=list(range(N_CORES)), trace=False
        )
    LAST_RESULT = res
    out = np.stack([res.results[b]["out"] for b in range(N_CORES)], axis=0)
    return out


# revision 21
# speedup vs baseline: 3610.2430x; 3610.2430x over previous
"""Axial attention (attention along W only) Trainium2 Bass kernel.

Reference computation (per batch b, row h):
    q = x @ wq; k = x @ wk; v = x @ wv          (x = inputs[b,h] in [W, C])
    s = q @ k^T ; p = softmax(s, axis=-1)
    out[b,h] = (p @ v) @ wv^T

Key algebraic fusion (collapses DIM entirely):
    s   = x @ (wq @ wk^T) @ x^T = x @ Wqk @ x^T
    out = p @ x @ (wv @ wv^T)   = p @ x @ Wvv
Wqk/Wvv are [C, C] host-precomputed weight products (one-time 256^3 matmuls).

Softmax without per-row max subtraction (constant bias -30 instead): scores are
~N(0, 10^2); fp32 exp is safe to |s| ~ 85, so overflow probability is nil and
the constant shift cancels exactly in the normalization.

Sharding: data-parallel over batch B=8 -> one batch per NeuronCore. Weights
replicated. Each core processes 128 independent (h) tiles of [W=128, C=256].

The shipped program is _build_program_v2: tiles are processed in PAIRS to
amortize per-instruction overheads (HWDGE setup ~625 ns/DMA, SWDGE ~1 us/DMA,
PE sequencer ~100 ns per LDWEIGHTS+MATMUL pair), which measured ~94-105 us
per core vs ~145 us for the single-tile version and ~430 us for a version
using on-chip DMA-transpose loads (xbar-mode serialization kills overlap --
the transposed copy of x is instead built on the host and loaded as part of
one fused 256 KB DMA per pair).

Per pair group (PE convention: matmul(out, lhsT, rhs) = lhsT.T @ rhs):
    one DMA: xx = [x_t0 | x_t1 | xT_t0 | xT_t1]   (bf16, [128, 4, 256])
    tT both tiles: sum_c Wqk[c,c2] xT[c,(t,i)]    4 MMs at N=256
    s[i,j], sT[j,i] per tile                      8 MMs at N=128 into one bank
    one exp over [s0|sT0|s1|sT1] (bias -30)       1 ACT op, bf16 out
    Z both tiles = rowsum of exp(s) halves        1 DVE strided reduce
    uT[c,i] = sum_j x[j,c] exp_sT[j,i]            2 MMs per tile
    o[i,c'] = sum_c uT[c,i] Wvv[c,c']             2 MMs per tile
    out = o * (1/Z[i])                            ACT copy+scale per tile
    one DMA store for the pair                    (fp32, 256 KB)
"""

import os

import numpy as np
import ml_dtypes

B, H, W, C = 8, 128, 128, 256
N_CORES = 8

_PROGRAM = None  # cached (nc,) build


def _build_program(
    n_htiles=H,
    repeat=1,
    pretransposed=True,
    work_bufs=4,
    psum_bufs=2,
    store_engine="gpsimd",
    out_bf16=False,
    psum_layout="pack3",
    loads_engine="sync",
):
    import concourse.bass as bass
    import concourse.mybir as mybir
    import concourse.tile as tile
    from concourse import bacc
    from concourse.bass import ts

    bf16 = mybir.dt.bfloat16
    f32 = mybir.dt.float32

    nc = bacc.Bacc(
        "TRN2",
        target_bir_lowering=False,
        debug=False,
        enable_asserts=False,
        num_devices=N_CORES,
    )

    x_d = nc.dram_tensor("x", [n_htiles, W, C], bf16, kind="ExternalInput").ap()
    if pretransposed:
        xt_d = nc.dram_tensor(
            "xt", [n_htiles, 128, 2, W], bf16, kind="ExternalInput"
        ).ap()
    wqk_d = nc.dram_tensor("wqk", [C, C], bf16, kind="ExternalInput").ap()
    wvv_d = nc.dram_tensor("wvv", [C, C], bf16, kind="ExternalInput").ap()
    out_dt = bf16 if out_bf16 else f32
    out_d = nc.dram_tensor("out", [n_htiles, W, C], out_dt, kind="ExternalOutput").ap()

    with tile.TileContext(nc) as tc:
        with (
            tc.tile_pool(name="singles", bufs=1) as singles,
            tc.tile_pool(name="work", bufs=work_bufs) as work,
            tc.tile_pool(name="psum", bufs=psum_bufs, space="PSUM") as psum,
        ):
            # Replicated weights, loaded once. Layout [p, chunk, col]:
            # wqk_s[p, o, c2] = Wqk[o*128 + p, c2]
            wqk_s = singles.tile([128, 2, C], bf16)
            nc.sync.dma_start(wqk_s, wqk_d.rearrange("(o p) c -> p o c", p=128))
            wvv_s = singles.tile([128, 2, C], bf16)
            nc.sync.dma_start(wvv_s, wvv_d.rearrange("(o p) c -> p o c", p=128))
            bias_s = singles.tile([128, 1], f32)
            nc.vector.memset(bias_s, -30.0)

            load_eng = nc.sync if loads_engine == "sync" else nc.gpsimd
            for h in range(n_htiles * repeat):
                h = h % n_htiles
                # ---- loads ----
                x_s = work.tile([W, C], bf16, tag="x")
                load_eng.dma_start(x_s, x_d[h])
                xT_s = work.tile([128, 2, W], bf16, tag="xT")
                if pretransposed:
                    load_eng.dma_start(xT_s, xt_d[h])
                else:
                    for c in range(2):
                        nc.sync.dma_start_transpose(
                            xT_s[:, c, :], x_d[h, :, ts(c, 128)]
                        )

                # ---- psum tiles (packed to stay within 8 banks) ----
                if psum_layout == "pack3":
                    ps_ts = psum.tile([128, 384], f32, tag="ps_ts")
                    ps_su = psum.tile([128, 384], f32, tag="ps_su")
                    ps_o = psum.tile([128, 256], f32, tag="ps_o")
                    tT_ps = ps_ts[:, 0:256]
                    s_ps = ps_ts[:, 256:384]
                    sT_ps = ps_su[:, 0:128]
                    uT_ps = ps_su[:, 128:384]
                    o_ps = ps_o
                else:  # pack2: two full banks, deeper bufs
                    ps_a = psum.tile([128, 512], f32, tag="ps_a")
                    ps_b = psum.tile([128, 512], f32, tag="ps_b")
                    tT_ps = ps_a[:, 0:256]
                    s_ps = ps_a[:, 256:384]
                    sT_ps = ps_a[:, 384:512]
                    uT_ps = ps_b[:, 0:256]
                    o_ps = ps_b[:, 256:512]

                # ---- tT = (x @ Wqk)^T : [c2, i] ----
                for j in range(2):  # c2 chunk
                    for o in range(2):  # contraction c chunk
                        nc.tensor.matmul(
                            tT_ps[:, ts(j, 128)],
                            lhsT=wqk_s[:, o, ts(j, 128)],
                            rhs=xT_s[:, o, :],
                            start=(o == 0),
                            stop=(o == 1),
                        )
                tT_s = work.tile([128, 256], bf16, tag="tT")
                nc.vector.tensor_copy(tT_s, tT_ps)

                # ---- s [i,j] and sT [j,i] ----
                for j in range(2):
                    nc.tensor.matmul(
                        s_ps,
                        lhsT=tT_s[:, ts(j, 128)],
                        rhs=xT_s[:, j, :],
                        start=(j == 0),
                        stop=(j == 1),
                    )
                for j in range(2):
                    nc.tensor.matmul(
                        sT_ps,
                        lhsT=xT_s[:, j, :],
                        rhs=tT_s[:, ts(j, 128)],
                        start=(j == 0),
                        stop=(j == 1),
                    )

                # ---- softmax pieces ----
                z_s = work.tile([128, 1], f32, tag="z")
                exps_scr = work.tile([128, 128], bf16, tag="exps")
                nc.scalar.activation(
                    out=exps_scr,
                    in_=s_ps,
                    func=mybir.ActivationFunctionType.Exp,
                    bias=bias_s,
                    scale=1.0,
                    accum_out=z_s,
                )
                rz_s = work.tile([128, 1], f32, tag="rz")
                nc.vector.reciprocal(rz_s, z_s)
                expsT_s = work.tile([128, 128], bf16, tag="expsT")
                nc.scalar.activation(
                    out=expsT_s,
                    in_=sT_ps,
                    func=mybir.ActivationFunctionType.Exp,
                    bias=bias_s,
                    scale=1.0,
                )

                # ---- uT [c, i] = x^T @ exp_sT ----
                for o in range(2):
                    nc.tensor.matmul(
                        uT_ps[:, ts(o, 128)],
                        lhsT=x_s[:, ts(o, 128)],
                        rhs=expsT_s,
                        start=True,
                        stop=True,
                    )
                uT_s = work.tile([128, 256], bf16, tag="uT")
                nc.vector.tensor_copy(uT_s, uT_ps)

                # ---- o [i, c'] = u @ Wvv ----
                for o in range(2):
                    nc.tensor.matmul(
                        o_ps,
                        lhsT=uT_s[:, ts(o, 128)],
                        rhs=wvv_s[:, o, :],
                        start=(o == 0),
                        stop=(o == 1),
                    )

                # ---- normalize + store ----
                out_s = work.tile([128, 256], out_dt, tag="out")
                nc.scalar.activation(
                    out=out_s,
                    in_=o_ps,
                    func=mybir.ActivationFunctionType.Identity,
                    bias=0.0,
                    scale=rz_s,
                )
                if store_engine == "gpsimd":
                    nc.gpsimd.dma_start(out_d[h], out_s)
                else:
                    nc.sync.dma_start(out_d[h], out_s)

    nc.finalize()
    return nc


def _build_program_v2(
    n_htiles=H,
    repeat=1,
    out_bf16=False,
    work_bufs=4,
    uo_bufs=4,
):
    """Pair-grouped variant: two h-tiles per group to amortize per-instruction
    overheads (one load DMA, batched tT matmuls at N=256, one merged exp over
    both tiles' s/sT, one reduce, one paired store)."""
    import concourse.mybir as mybir
    import concourse.tile as tile
    from concourse import bacc
    from concourse.bass import ts

    bf16 = mybir.dt.bfloat16
    f32 = mybir.dt.float32
    assert n_htiles % 2 == 0
    n_groups = n_htiles // 2

    nc = bacc.Bacc(
        "TRN2",
        target_bir_lowering=False,
        debug=False,
        enable_asserts=False,
        num_devices=N_CORES,
    )

    # xx[g, p, q, :]: q=0,1 -> x rows of tiles (2g, 2g+1) ([w=p, c]);
    #                q=2,3 -> xT of tiles (2g, 2g+1) ([c_in=p, (o,w)])
    xx_d = nc.dram_tensor(
        "xx", [n_groups, 128, 4, 256], bf16, kind="ExternalInput"
    ).ap()
    wqk_d = nc.dram_tensor("wqk", [C, C], bf16, kind="ExternalInput").ap()
    wvv_d = nc.dram_tensor("wvv", [C, C], bf16, kind="ExternalInput").ap()
    out_dt = bf16 if out_bf16 else f32
    out_d = nc.dram_tensor("out", [n_htiles, W, C], out_dt, kind="ExternalOutput").ap()

    with tile.TileContext(nc) as tc:
        with (
            tc.tile_pool(name="singles", bufs=1) as singles,
            tc.tile_pool(name="work", bufs=work_bufs) as work,
            tc.tile_pool(name="psum", bufs=2, space="PSUM") as psum,
        ):
            wqk_s = singles.tile([128, 2, C], bf16)
            nc.sync.dma_start(wqk_s, wqk_d.rearrange("(o p) c -> p o c", p=128))
            wvv_s = singles.tile([128, 2, C], bf16)
            nc.sync.dma_start(wvv_s, wvv_d.rearrange("(o p) c -> p o c", p=128))
            bias_s = singles.tile([128, 1], f32)
            nc.vector.memset(bias_s, -30.0)

            for g in range(n_groups * repeat):
                g = g % n_groups
                xx_s = work.tile([128, 4, 256], bf16, tag="xx")
                nc.sync.dma_start(xx_s, xx_d[g])

                # views into the combined load
                def x_t(t):
                    return xx_s[:, t, :]          # [w, c]

                def xT_t(t, o):
                    return xx_s[:, 2 + t, ts(o, 128)]  # [c_in, w] chunk o

                # ---- tT for both tiles: batched rhs [p, (t, w)] = 256 ----
                tT2_ps = psum.tile([128, 512], f32, tag="tT2")
                for j in range(2):
                    for o in range(2):
                        nc.tensor.matmul(
                            tT2_ps[:, ts(j, 256)],
                            lhsT=wqk_s[:, o, ts(j, 128)],
                            rhs=xx_s[:, 2:4, ts(o, 128)],  # [p, t, w] = 256 free
                            start=(o == 0),
                            stop=(o == 1),
                        )
                tT2_s = work.tile([128, 512], bf16, tag="tT2s")
                nc.vector.tensor_copy(tT2_s, tT2_ps)

                # tT slice for tile t, chunk j: [c2-in-chunk, i]
                def tT_t(t, j):
                    return tT2_s[:, j * 256 + t * 128 : j * 256 + t * 128 + 128]

                # ---- s and sT for both tiles into one bank ----
                # layout: [s_t0 | sT_t0 | s_t1 | sT_t1] each 128 cols
                ss2_ps = psum.tile([128, 512], f32, tag="ss2")
                for t in range(2):
                    for j in range(2):
                        nc.tensor.matmul(
                            ss2_ps[:, t * 256 : t * 256 + 128],
                            lhsT=tT_t(t, j),
                            rhs=xT_t(t, j),
                            start=(j == 0),
                            stop=(j == 1),
                        )
                    for j in range(2):
                        nc.tensor.matmul(
                            ss2_ps[:, t * 256 + 128 : t * 256 + 256],
                            lhsT=xT_t(t, j),
                            rhs=tT_t(t, j),
                            start=(j == 0),
                            stop=(j == 1),
                        )

                # ---- single exp over both tiles' s and sT ----
                expscr = work.tile([128, 512], bf16, tag="expscr")
                nc.scalar.activation(
                    out=expscr,
                    in_=ss2_ps,
                    func=mybir.ActivationFunctionType.Exp,
                    bias=bias_s,
                    scale=1.0,
                )
                # Z for both tiles in one reduce over the exp(s) halves
                z2 = work.tile([128, 2], f32, tag="z2")
                nc.vector.reduce_sum(
                    out=z2,
                    in_=expscr.rearrange("p (t c) -> p t c", t=4)[:, 0::2, :],
                    axis=mybir.AxisListType.X,
                )
                rz2 = work.tile([128, 2], f32, tag="rz2")
                nc.vector.reciprocal(rz2, z2)

                uT_s = work.tile([128, 512], bf16, tag="uTs")
                out2 = work.tile([128, 2, 256], out_dt, tag="out2")
                for t in range(2):
                    uo_ps = psum.tile([128, 512], f32, tag="uo", bufs=uo_bufs)
                    expsT = expscr[:, t * 256 + 128 : t * 256 + 256]
                    for o in range(2):
                        nc.tensor.matmul(
                            uo_ps[:, ts(o, 128)],
                            lhsT=x_t(t)[:, ts(o, 128)],
                            rhs=expsT,
                            start=True,
                            stop=True,
                        )
                    # copy uT to SBUF; alternate engines to balance ACT/DVE
                    if t == 0:
                        nc.scalar.copy(uT_s[:, 0:256], uo_ps[:, 0:256])
                    else:
                        nc.vector.tensor_copy(uT_s[:, 256:512], uo_ps[:, 0:256])
                    for o in range(2):
                        nc.tensor.matmul(
                            uo_ps[:, 256:512],
                            lhsT=uT_s[:, t * 256 + o * 128 : t * 256 + o * 128 + 128],
                            rhs=wvv_s[:, o, :],
                            start=(o == 0),
                            stop=(o == 1),
                        )
                    nc.scalar.activation(
                        out=out2[:, t, :],
                        in_=uo_ps[:, 256:512],
                        func=mybir.ActivationFunctionType.Identity,
                        bias=0.0,
                        scale=rz2[:, t : t + 1],
                    )

                nc.gpsimd.dma_start(
                    out_d[2 * g : 2 * g + 2].rearrange("t w c -> w t c"), out2
                )

    nc.finalize()
    return nc


def _build_program_v3(
    n_htiles=H,
    repeat=1,
    work_bufs=4,
    uo_bufs=4,
    tt_bufs=2,
    ss_bufs=1,
):
    """Quad-grouped variant: four h-tiles per group. One 512KB load and one
    512KB store per group, tT matmuls at N=512 (one LDWEIGHTS per Wqk block
    per group), two merged exps (s-block, sT-block), one contiguous Z-reduce.
    PSUM banks: tT4 x tt_bufs + (ssA+ssB) x ss_bufs + uo x uo_bufs <= 8.
    """
    import concourse.mybir as mybir
    import concourse.tile as tile
    from concourse import bacc
    from concourse.bass import ts

    bf16 = mybir.dt.bfloat16
    f32 = mybir.dt.float32
    assert n_htiles % 4 == 0
    n_groups = n_htiles // 4

    nc = bacc.Bacc(
        "TRN2",
        target_bir_lowering=False,
        debug=False,
        enable_asserts=False,
        num_devices=N_CORES,
    )

    # xx[g, p, q, :]: q=0..3 -> x rows of tiles 4g..4g+3 ([w=p, c]);
    #                q=4..7 -> xT of tiles 4g..4g+3 ([c_in=p, (o,w)])
    xx_d = nc.dram_tensor(
        "xx", [n_groups, 128, 8, 256], bf16, kind="ExternalInput"
    ).ap()
    wqk_d = nc.dram_tensor("wqk", [C, C], bf16, kind="ExternalInput").ap()
    wvv_d = nc.dram_tensor("wvv", [C, C], bf16, kind="ExternalInput").ap()
    out_d = nc.dram_tensor("out", [n_htiles, W, C], f32, kind="ExternalOutput").ap()

    with tile.TileContext(nc) as tc:
        with (
            tc.tile_pool(name="singles", bufs=1) as singles,
            tc.tile_pool(name="work", bufs=work_bufs) as work,
            tc.tile_pool(name="psum", bufs=2, space="PSUM") as psum,
        ):
            wqk_s = singles.tile([128, 2, C], bf16)
            nc.sync.dma_start(wqk_s, wqk_d.rearrange("(o p) c -> p o c", p=128))
            wvv_s = singles.tile([128, 2, C], bf16)
            nc.sync.dma_start(wvv_s, wvv_d.rearrange("(o p) c -> p o c", p=128))
            bias_s = singles.tile([128, 1], f32)
            nc.vector.memset(bias_s, -30.0)

            for g in range(n_groups * repeat):
                g = g % n_groups
                xx_s = work.tile([128, 8, 256], bf16, tag="xx")
                nc.sync.dma_start(xx_s, xx_d[g])

                def x_t(t):
                    return xx_s[:, t, :]               # [w, c]

                def xT_t(t, o):
                    return xx_s[:, 4 + t, ts(o, 128)]  # [c_in, w] chunk o

                # ---- tT for all 4 tiles: rhs [p, (t, w)] = 512 free ----
                tT4_s = work.tile([128, 2, 512], bf16, tag="tT4s")
                for j in range(2):
                    tT4_ps = psum.tile([128, 512], f32, tag="tT4", bufs=tt_bufs)
                    for o in range(2):
                        nc.tensor.matmul(
                            tT4_ps,
                            lhsT=wqk_s[:, o, ts(j, 128)],
                            rhs=xx_s[:, 4:8, ts(o, 128)],  # [p, 4, 128]
                            start=(o == 0),
                            stop=(o == 1),
                        )
                    nc.vector.tensor_copy(tT4_s[:, j, :], tT4_ps)

                def tT_t(t, j):
                    return tT4_s[:, j, ts(t, 128)]     # [c2-in-chunk, i]

                # ---- s block (bank A) and sT block (bank B) ----
                ssA = psum.tile([128, 512], f32, tag="ssA", bufs=ss_bufs)
                ssB = psum.tile([128, 512], f32, tag="ssB", bufs=ss_bufs)
                for t in range(4):
                    for j in range(2):
                        nc.tensor.matmul(
                            ssA[:, ts(t, 128)],
                            lhsT=tT_t(t, j),
                            rhs=xT_t(t, j),
                            start=(j == 0),
                            stop=(j == 1),
                        )
                for t in range(4):
                    for j in range(2):
                        nc.tensor.matmul(
                            ssB[:, ts(t, 128)],
                            lhsT=xT_t(t, j),
                            rhs=tT_t(t, j),
                            start=(j == 0),
                            stop=(j == 1),
                        )

                # ---- exps ----
                expsA = work.tile([128, 512], bf16, tag="expsA")  # exp(s)
                nc.scalar.activation(
                    out=expsA, in_=ssA,
                    func=mybir.ActivationFunctionType.Exp,
                    bias=bias_s, scale=1.0,
                )
                expsB = work.tile([128, 512], bf16, tag="expsB")  # exp(sT)
                nc.scalar.activation(
                    out=expsB, in_=ssB,
                    func=mybir.ActivationFunctionType.Exp,
                    bias=bias_s, scale=1.0,
                )
                z4 = work.tile([128, 4], f32, tag="z4")
                nc.vector.reduce_sum(
                    out=z4,
                    in_=expsA.rearrange("p (t c) -> p t c", t=4),
                    axis=mybir.AxisListType.X,
                )
                rz4 = work.tile([128, 4], f32, tag="rz4")
                nc.vector.reciprocal(rz4, z4)

                uT_s = work.tile([128, 4, 256], bf16, tag="uTs")
                out4 = work.tile([128, 4, 256], f32, tag="out4")
                for t in range(4):
                    uo_ps = psum.tile([128, 512], f32, tag="uo", bufs=uo_bufs)
                    expsT = expsB[:, ts(t, 128)]
                    for o in range(2):
                        nc.tensor.matmul(
                            uo_ps[:, ts(o, 128)],
                            lhsT=x_t(t)[:, ts(o, 128)],
                            rhs=expsT,
                            start=True,
                            stop=True,
                        )
                    if t % 2 == 0:
                        nc.scalar.copy(uT_s[:, t, :], uo_ps[:, 0:256])
                    else:
                        nc.vector.tensor_copy(uT_s[:, t, :], uo_ps[:, 0:256])
                    for o in range(2):
                        nc.tensor.matmul(
                            uo_ps[:, 256:512],
                            lhsT=uT_s[:, t, ts(o, 128)],
                            rhs=wvv_s[:, o, :],
                            start=(o == 0),
                            stop=(o == 1),
                        )
                    nc.scalar.activation(
                        out=out4[:, t, :],
                        in_=uo_ps[:, 256:512],
                        func=mybir.ActivationFunctionType.Identity,
                        bias=0.0,
                        scale=rz4[:, t : t + 1],
                    )

                nc.gpsimd.dma_start(
                    out_d[4 * g : 4 * g + 4].rearrange("t w c -> w t c"), out4
                )

    nc.finalize()
    return nc


def _prep_in_maps_v3(inputs, wq, wk, wv):
    bf16 = ml_dtypes.bfloat16
    wqk = (wq.astype(np.float32) @ wk.astype(np.float32).T).astype(bf16)
    wvv = (wv.astype(np.float32) @ wv.astype(np.float32).T).astype(bf16)
    x_bf = np.ascontiguousarray(inputs.astype(bf16))
    nb, nh, nw, c = x_bf.shape
    ng = nh // 4
    xt = x_bf.transpose(0, 1, 3, 2).reshape(nb, nh, 2, 128, nw)
    xt = xt.transpose(0, 1, 3, 2, 4).reshape(nb, nh, 128, 256)
    xx = np.empty((nb, ng, 128, 8, 256), dtype=bf16)
    for t in range(4):
        xx[:, :, :, t] = x_bf[:, t::4]
        xx[:, :, :, 4 + t] = xt[:, t::4]
    return [
        {"xx": np.ascontiguousarray(xx[b]), "wqk": wqk, "wvv": wvv}
        for b in range(nb)
    ]


def _prep_in_maps_v2(inputs, wq, wk, wv):
    bf16 = ml_dtypes.bfloat16
    wqk = (wq.astype(np.float32) @ wk.astype(np.float32).T).astype(bf16)
    wvv = (wv.astype(np.float32) @ wv.astype(np.float32).T).astype(bf16)
    x_bf = np.ascontiguousarray(inputs.astype(bf16))
    nb, nh, nw, c = x_bf.shape
    ng = nh // 2
    # xt[b,h,p,o,w] = x[b,h,w,o*128+p]
    xt = x_bf.transpose(0, 1, 3, 2).reshape(nb, nh, 2, 128, nw)
    xt = xt.transpose(0, 1, 3, 2, 4).reshape(nb, nh, 128, 256)
    xx = np.empty((nb, ng, 128, 4, 256), dtype=bf16)
    xx[:, :, :, 0] = x_bf[:, 0::2].transpose(0, 1, 2, 3)[:, :, :, :].reshape(
        nb, ng, nw, c
    )
    xx[:, :, :, 1] = x_bf[:, 1::2].reshape(nb, ng, nw, c)
    xx[:, :, :, 2] = xt[:, 0::2]
    xx[:, :, :, 3] = xt[:, 1::2]
    return [
        {"xx": np.ascontiguousarray(xx[b]), "wqk": wqk, "wvv": wvv}
        for b in range(nb)
    ]


def _get_program():
    global _PROGRAM
    if _PROGRAM is None:
        _PROGRAM = _build_program_v2()
    return _PROGRAM


LAST_RESULT = None  # BassKernelResults of the most recent kernel() call


def _prep_in_maps(inputs, wq, wk, wv, pretransposed=True):
    bf16 = ml_dtypes.bfloat16
    wqk = (wq.astype(np.float32) @ wk.astype(np.float32).T).astype(bf16)
    wvv = (wv.astype(np.float32) @ wv.astype(np.float32).T).astype(bf16)
    x_bf = np.ascontiguousarray(inputs.astype(bf16))
    nb, nh, nw, c = x_bf.shape
    in_maps = [{"x": x_bf[b], "wqk": wqk, "wvv": wvv} for b in range(nb)]
    if pretransposed:
        # xt[b,h,p,o,w] = x[b,h,w,o*128+p]; DRAM layout [H, 128, 2, W]
        xt = x_bf.transpose(0, 1, 3, 2).reshape(nb, nh, 2, 128, nw)
        xt = np.ascontiguousarray(xt.transpose(0, 1, 3, 2, 4))
        for b in range(nb):
            in_maps[b]["xt"] = xt[b]
    return in_maps


def kernel(inputs, wq, wk, wv):
    """Full-input entry point: inputs [8,128,128,256] f32, wq/wk/wv [256,256] f32.
    Returns [8,128,128,256] f32."""
    global LAST_RESULT
    import tempfile

    # The libneuronxla compile cache keys on the HLO module only (the BIR
    # travels out-of-band), so different Bass programs with identical I/O
    # shapes would collide. Use a fresh cache dir to guarantee this program
    # compiles (walrus is sub-second for this kernel).
    os.environ.setdefault(
        "NEURON_COMPILE_CACHE_URL", tempfile.mkdtemp(prefix="ncc-axial-")
    )
    from concourse.bass_utils import run_bass_kernel_spmd

    in_maps = _prep_in_maps_v2(inputs, wq, wk, wv)

    nc = _get_program()
    trace = bool(int(os.environ.get("AXIAL_TRACE", "0")))
    try:
        res = run_bass_kernel_spmd(
            nc,
            in_maps,
            core_ids=list(range(N_CORES)),
            trace=trace,
        )
    except ModuleNotFoundError:
        if not trace:
            raise
        res = run_bass_kernel_spmd(
            nc, in_maps, core_ids=list(range(N_CORES)), trace=False
        )
    LAST_RESULT = res
    out = np.stack([res.results[b]["out"] for b in range(N_CORES)], axis=0)
    return out
